# revision 30
# baseline (speedup 1.0000x reference)
"""Trainium2 Bass kernel for a pre-LN transformer block (dense_transformer).

Shapes (hardcoded): B=16, N=1024, D=768, H=12, HD=64, HID=3072.
Sharding: data-parallel over batch across 8 NeuronCores (2 batch elements,
i.e. 2048 tokens, per core). Weights replicated; no collectives.

Per-core dataflow (activations kept on-chip, bf16 matmuls / fp32 stats):
  LN1 (token-major, fp32)   -> h1 bf16, PE-transposed to h1T [D, T]
  qkT [1536, T] = W_qk^T-stationary matmuls (LN gamma + 1/sqrt(hd) folded
                  into weights host-side)
  v_aug [T, 12*(64+1)]      = per-head V columns + a ones column (the ones
                  column makes the AV matmul also produce the softmax
                  denominator)
  per (batch, head):  scoresT[krow, qrow] = k^T.T @ q^T  (K=64 contraction)
                      attnT = Exp(scoresT)  (ScalarE, psum->sbuf bf16; exp is
                              max-free: |scores| is small by construction)
                      out^T[65, qrow] = [v | 1]^T-stationary @ attnT
                      row 64 = denominator -> fast reciprocal -> gpsimd
                      partition broadcast -> normalize rows 0..63 -> attn_oT
  proj (token-major out) + proj_b + residual x  -> y (fp32, SBUF)
  LN2 -> h2T;  fc1 (out-feature-major) + bias + exact Gelu -> gT
  fc2 (token-major out) + fc2_b + residual y -> output

Scheduling structure: one shared 4-slot PSUM pool for every matmul /
transpose (no cross-phase pool barriers); emission order
A B C(b=0) C(b=1) proj/LN2(b=0) proj/LN2(b=1) MLP so the Tile scheduler
backfills PE with proj work while attention is ScalarE(exp)-bound.
SBUF pools are two-sided stacks with per-batch y / h2T split so residual
buffers for batch 0 exist during batch-1 attention.
"""

import numpy as np
import ml_dtypes

B, N, D = 16, 1024, 768
H = 12
HD = D // H
HID = 4 * D
EPS = 1e-6
NCORES = 8
BC = B // NCORES          # batch elements per core
T = BC * N                # tokens per core (2048)
NTT = T // 128            # token tiles (16)
KC = D // 128             # contraction chunks over D (6)
NTW = T // 512            # token windows of 512 (4)
NKT = N // 128            # key-row tiles per batch element (8)
NQW = N // 512            # query windows per batch element (2)
NOF1 = HID // 128         # fc1 output-feature tiles (24)

_PROG_CACHE = {}

# sim-vs-HW bisection flags
F_RECIP_FAST = True    # reciprocal_approx_fast vs exact reciprocal
F_GP_OPS = True        # gpsimd tensor_add/memset offload
F_ANY_COPY = True      # nc.any transpose copies (vs nc.vector)
F_ACT_SCATTER = True   # ACT Copy with strided 3-D dst in V scatter

# act_info.json set index for natural_log_exp_and_others (ln + exp + copy in
# one table set). Explicitly pre-loading it stops the compiler's per-function
# greedy table choice (ln -> natural_log, exp -> exp_and_others) from
# thrashing ACT_TABLE_LOADs (1283ns each) on every LN rstd.
ACT_SET_LN_EXP = 6
F_EXPLICIT_ACT_LOAD = True


def _build_program(has_qkv_bias):
    import concourse.bass as bass
    import concourse.mybir as mybir
    import concourse.tile as tile
    from concourse import bacc
    from concourse.masks import make_identity
    from contextlib import ExitStack

    F32 = mybir.dt.float32
    BF = mybir.dt.bfloat16
    AF = mybir.ActivationFunctionType
    ALU = mybir.AluOpType

    nc = bacc.Bacc("TRN2", target_bir_lowering=False, debug=False,
                   num_devices=NCORES)

    x_in = nc.dram_tensor("x", [T, D], F32, kind="ExternalInput").ap()
    qk_wT = nc.dram_tensor("qk_wT", [D, 2 * D], BF, kind="ExternalInput").ap()
    v_wT = nc.dram_tensor("v_wT", [D, D], BF, kind="ExternalInput").ap()
    proj_wT = nc.dram_tensor("proj_wT", [D, D], BF, kind="ExternalInput").ap()
    fc1_wT = nc.dram_tensor("fc1_wT", [D, HID], BF, kind="ExternalInput").ap()
    fc2_wT = nc.dram_tensor("fc2_wT", [HID, D], BF, kind="ExternalInput").ap()
    proj_b = nc.dram_tensor("proj_b", [D], F32, kind="ExternalInput").ap()
    fc1_b = nc.dram_tensor("fc1_b", [HID], F32, kind="ExternalInput").ap()
    fc2_b = nc.dram_tensor("fc2_b", [D], F32, kind="ExternalInput").ap()
    if has_qkv_bias:
        qk_bias = nc.dram_tensor("qk_b", [2 * D], F32, kind="ExternalInput").ap()
        v_bias = nc.dram_tensor("v_b", [D], F32, kind="ExternalInput").ap()
    y_out = nc.dram_tensor("y", [T, D], F32, kind="ExternalOutput").ap()

    y_scr = nc.dram_tensor("y_scr", [T, D], F32).ap()

    with tile.TileContext(nc) as tc, ExitStack() as ctx:
        # one PSUM pool for everything:
        #   "sc" 2 x [128,1024]f32 (2 banks each) = scores pairs + fc1
        #   "av" 2 x [65,512]f32   (1 bank each)  = AV accumulators
        #   "mm" 2 x [128,512]f32  (1 bank each)  = qkv/proj/fc2/transposes
        # -> 8 banks total
        psum = tc.alloc_tile_pool(name="psum", bufs=2, space="PSUM")

        # Pin the combined ln+exp table set once, before any activation; all
        # LN rstd (Ln+Exp) and attention exp then run switch-free. Gelu (MLP
        # tail) is the only later switch.
        if F_EXPLICIT_ACT_LOAD:
            ld_act = mybir.InstLoadActFuncSet(
                name=nc.get_next_instruction_name(),
                act_func_set_id=ACT_SET_LN_EXP, ins=[], outs=[])
            nc.scalar.add_instruction(ld_act)

        singles = ctx.enter_context(tc.tile_pool(name="singles", bufs=1,
                                                 side="left"))
        ident = singles.tile([128, 128], BF)
        make_identity(nc, ident)
        eps_t = singles.tile([128, 1], F32)
        nc.vector.memset(eps_t, EPS)
        fc1b_sb = singles.tile([128, NOF1], F32)

        last_rstd = [None]

        def bcast128(ap1d):
            return bass.AP(tensor=ap1d.tensor, offset=ap1d.offset,
                           ap=[[0, 128]] + list(ap1d.ap))

        projb_sb = singles.tile([128, D], F32)
        fc2b_sb = singles.tile([128, D], F32)
        if has_qkv_bias:
            qkb_sb = singles.tile([128, 2 * D // 128], F32)
            vb_sb = singles.tile([128, D], F32)

        # batch-0 attention output + batch-0 h2T live low on the left stack
        oT0_pool = tc.alloc_tile_pool(name="oT0", bufs=KC, side="left")
        h2T0_pool = tc.alloc_tile_pool(name="h2T0", bufs=KC, side="left")
        attn_oT = [[oT0_pool.tile([128, N], BF, tag="oT0", name=f"oT0_{i}")
                    for i in range(KC)], None]
        h2T_b = [[h2T0_pool.tile([128, N], BF, tag="h2T0", name=f"h2T0_{i}")
                  for i in range(KC)], None]

        h1T_pool = tc.alloc_tile_pool(name="h1T", bufs=KC, side="left")
        h1T = [h1T_pool.tile([128, T], BF, tag="h1T", name=f"h1T{i}")
               for i in range(KC)]

        def layernorm_tile(src, tmp_pool, hbf_pool, h_T, tt, tag):
            """LN over one [128, D] fp32 tile -> bf16 + PE-transpose into h_T."""
            stats = tmp_pool.tile([128, 2, 6], F32, tag=f"st{tag}",
                                  name=f"st{tag}_{tt}")
            for sg in range(2):
                nc.vector.bn_stats(stats[:, sg, :], src[:, sg * 384:(sg + 1) * 384])
            mv = tmp_pool.tile([128, 2], F32, tag=f"mv{tag}", name=f"mv{tag}_{tt}")
            nc.vector.bn_aggr(mv, stats)
            rstd = tmp_pool.tile([128, 1], F32, tag=f"rs{tag}", name=f"rs{tag}_{tt}")
            # rstd = (var+eps)^-1/2 = exp(-0.5*ln(var+eps)); Ln and Exp share
            # one ACT table set, unlike Sqrt
            nc.scalar.activation(rstd, mv[:, 1:2], AF.Ln, bias=eps_t)
            nc.scalar.activation(rstd, rstd, AF.Exp, scale=-0.5)
            negmr = tmp_pool.tile([128, 1], F32, tag=f"nm{tag}", name=f"nm{tag}_{tt}")
            nc.vector.tensor_scalar(negmr, mv[:, 0:1], rstd, -1.0,
                                    ALU.mult, ALU.mult)
            hbf = hbf_pool.tile([128, D], BF, tag=f"h{tag}", name=f"h{tag}_{tt}")
            nc.vector.tensor_scalar(hbf, src, rstd, negmr, ALU.mult, ALU.add)
            last_rstd[0] = rstd
            for kc in range(KC):
                pt = psum.tile([128, 128], BF, tag="mm", name=f"pt{tag}_{tt}_{kc}")
                nc.tensor.transpose(pt, hbf[:, kc * 128:(kc + 1) * 128], ident)
                # Copy lives in every ACT table set (no table-switch cost);
                # nc.any lets the scheduler route to whichever of ACT/DVE is
                # idle in this stretch.
                if F_ANY_COPY:
                    nc.any.tensor_copy(h_T[kc][:, tt * 128:(tt + 1) * 128], pt)
                else:
                    nc.vector.tensor_copy(h_T[kc][:, tt * 128:(tt + 1) * 128], pt)

        # qkv weights (left, die after phase_b(1)); DMAs emitted inside
        # phase A right after the first x group so they overlap LN1
        wqk_pool = tc.alloc_tile_pool(name="wqk", bufs=1, side="left")
        wv_pool = tc.alloc_tile_pool(name="wv", bufs=1, side="left")
        qkw_big = wqk_pool.tile([128, KC * 2 * D], BF, name="wqk_big")
        vw_big = wv_pool.tile([128, KC * D], BF, name="wv_big")
        qkw_sb = [qkw_big[:, kc * 2 * D:(kc + 1) * 2 * D] for kc in range(KC)]
        vw_sb = [vw_big[:, kc * D:(kc + 1) * D] for kc in range(KC)]

        # ---------- Phase A: LN1 + transpose (x loaded 2 tiles per DMA) ----
        with tc.tile_pool(name="ln_in", bufs=4, side="right") as xin_pool, \
             tc.tile_pool(name="ln_tmp", bufs=8, side="right") as tmp_pool, \
             tc.tile_pool(name="ln_out", bufs=4, side="right") as hbf_pool:
            for g in range(NTT // 2):
                xg = xin_pool.tile([128, 2, D], F32, tag="xt", name=f"xg{g}")
                # alternate issue engines: SWDGE first-byte setup (~1.2us per
                # dma_start) serializes per issuing engine
                eng = nc.sync if g % 2 == 0 else nc.gpsimd
                eng.dma_start(
                    out=xg,
                    in_=x_in[g * 256:(g + 1) * 256, :]
                    .rearrange("(t p) d -> p t d", p=128))
                if g == 4:
                    nc.sync.dma_start(
                        out=qkw_big.rearrange("p (c d) -> p c d", d=2 * D),
                        in_=qk_wT.rearrange("(c p) d -> p c d", p=128))
                    nc.sync.dma_start(
                        out=vw_big.rearrange("p (c d) -> p c d", d=D),
                        in_=v_wT.rearrange("(c p) d -> p c d", p=128))
                for t in range(2):
                    layernorm_tile(xg[:, t, :], tmp_pool, hbf_pool, h1T,
                                   g * 2 + t, "a")
        nc.sync.dma_start(out=fc1b_sb, in_=fc1_b.rearrange("(a p) -> p a", p=128))
        nc.sync.dma_start(out=projb_sb, in_=bcast128(proj_b))
        nc.sync.dma_start(out=fc2b_sb, in_=bcast128(fc2_b))
        if has_qkv_bias:
            nc.sync.dma_start(out=qkb_sb,
                              in_=qk_bias.rearrange("(a p) -> p a", p=128))
            nc.sync.dma_start(out=vb_sb, in_=bcast128(v_bias))

        # attention-side pools (right stack); per-batch qkT / v_aug so the
        # batch-0 halves release as soon as batch-0 attention has read them
        at_pool = tc.alloc_tile_pool(name="attnT", bufs=20, side="right")
        rec_pool = tc.alloc_tile_pool(name="rec", bufs=2, side="right")
        recb_pool = tc.alloc_tile_pool(name="recb", bufs=2, side="right")
        qkT1_pool = tc.alloc_tile_pool(name="qkT1", bufs=12, side="right")
        vaug1_pool = tc.alloc_tile_pool(name="vaug1", bufs=NKT, side="right")
        qkT0_pool = tc.alloc_tile_pool(name="qkT0", bufs=12, side="right")
        vaug0_pool = tc.alloc_tile_pool(name="vaug0", bufs=NKT, side="right")
        qkT_b = [
            [qkT0_pool.tile([128, N], BF, tag="qkT0", name=f"qkT0_{i}")
             for i in range(12)],
            [qkT1_pool.tile([128, N], BF, tag="qkT1", name=f"qkT1_{i}")
             for i in range(12)],
        ]
        v_aug = [vaug0_pool.tile([128, H * (HD + 1)], BF, tag="vaug0",
                                 name=f"vaug{i}") for i in range(NKT)]
        v_aug += [vaug1_pool.tile([128, H * (HD + 1)], BF, tag="vaug1",
                                  name=f"vaug{NKT + i}") for i in range(NKT)]

        def phase_b(b):
            """QK^T and V for batch b's token windows."""
            for of in range(12):
                for w in range(2):
                    tw = 2 * b + w
                    ps = psum.tile([128, 512], F32, tag="mm", name=f"qk{of}_{tw}")
                    for kc in range(KC):
                        nc.tensor.matmul(
                            ps,
                            lhsT=qkw_sb[kc][:, of * 128:(of + 1) * 128],
                            rhs=h1T[kc][:, tw * 512:(tw + 1) * 512],
                            start=(kc == 0), stop=(kc == KC - 1))
                    dst = qkT_b[b][of][:, w * 512:(w + 1) * 512]
                    if has_qkv_bias:
                        nc.scalar.activation(dst, ps, AF.Copy,
                                             bias=qkb_sb[:, of:of + 1])
                    else:
                        nc.any.tensor_copy(dst, ps)
            for tt in range(b * NKT, (b + 1) * NKT):
                for n0, nsz in ((0, 512), (512, 256)):
                    ps = psum.tile([128, nsz], F32, tag="mm", name=f"v{tt}_{n0}")
                    for kc in range(KC):
                        nc.tensor.matmul(
                            ps,
                            lhsT=h1T[kc][:, tt * 128:(tt + 1) * 128],
                            rhs=vw_sb[kc][:, n0:n0 + nsz],
                            start=(kc == 0), stop=(kc == KC - 1))
                    nh = nsz // HD
                    h0 = n0 // HD
                    dst = v_aug[tt][:, h0 * (HD + 1):(h0 + nh) * (HD + 1)] \
                        .rearrange("p (n c) -> p n c", c=HD + 1)[:, :, 0:HD]
                    psr = ps.rearrange("p (n c) -> p n c", c=HD)
                    if has_qkv_bias:
                        src_b = vb_sb[:, n0:n0 + nsz] \
                            .rearrange("p (n c) -> p n c", c=HD)
                        nc.vector.tensor_add(dst, psr, src_b)
                    elif F_ACT_SCATTER:
                        nc.any.tensor_copy(dst, psr)
                    else:
                        nc.vector.tensor_copy(dst, psr)
                ones_dst = v_aug[tt].rearrange(
                    "p (n c) -> p n c", c=HD + 1)[:, :, HD:HD + 1]
                if F_GP_OPS:
                    nc.gpsimd.memset(ones_dst, 1.0)
                else:
                    nc.vector.memset(ones_dst, 1.0)

        def attention(b, filler=None):
            """Attention for batch b, one head PAIR (2p, 2p+1) at a time.

            Scores for the pair are row-tiled on the PE: head 2p's K/Q live
            in partitions 0-63 of the of-tiles, head 2p+1's in 64-127, so the
            two K=64 matmuls land in PE row-groups (0,0)/(64,0) and run
            CONCURRENTLY (2x). Both heads' scores for one 512-token q window
            share one [128, 1024] PSUM tile (cols 0:512 = head A, 512:1024 =
            head B) so the pair gates on a single slot event and issues
            back-to-back.

            qw-major score order: all 8 key tiles of q-window 0 first, so the
            AV for qw0 becomes ready after 8 exps instead of 15 and overlaps
            the qw1 scores.

            `filler(p)` is called once per pair to emit interleaved backfill
            work (e.g. proj tiles of the other batch) at matching priority so
            the static per-engine streams alternate attention and backfill.
            """
            for p in range(H // 2):
                q_t = qkT_b[b][p]
                k_t = qkT_b[b][6 + p]
                # ats[kt][qw]: [128 krows, 1024] = (head A qw | head B qw)
                ats = [[at_pool.tile([128, N], BF, tag="at",
                                     name=f"at{b}_{p}_{kt}_{qw}")
                        for qw in range(NQW)] for kt in range(NKT)]
                if filler is not None:
                    filler(p)
                for qw in range(NQW):
                    for kt in range(NKT):
                        ps = psum.tile([128, N], F32, tag="sc",
                                       name=f"sc{b}_{p}_{kt}_{qw}")
                        for r0 in (0, HD):
                            nc.tensor.matmul(
                                ps[:, (r0 // HD) * 512:(r0 // HD) * 512 + 512],
                                lhsT=k_t[r0:r0 + HD, kt * 128:(kt + 1) * 128],
                                rhs=q_t[r0:r0 + HD, qw * 512:(qw + 1) * 512],
                                start=True, stop=True)
                        nc.scalar.activation(ats[kt][qw], ps, AF.Exp)
                o_t = attn_oT[b][p]
                for qw in range(NQW):
                    for hh in range(2):
                        h = 2 * p + hh
                        r0 = hh * HD
                        pav = psum.tile([HD + 1, 512], F32, tag="av",
                                        name=f"av{b}_{h}_{qw}")
                        for kt in range(NKT):
                            nc.tensor.matmul(
                                pav,
                                lhsT=v_aug[b * NKT + kt][
                                    :, h * (HD + 1):(h + 1) * (HD + 1)],
                                rhs=ats[kt][qw][:, hh * 512:hh * 512 + 512],
                                start=(kt == 0), stop=(kt == NKT - 1))
                        rec = rec_pool.tile([1, 512], F32, tag="rec",
                                            name=f"rec{b}_{h}_{qw}")
                        if F_RECIP_FAST:
                            den = rec_pool.tile([1, 512], F32, tag="den",
                                                name=f"den{b}_{h}_{qw}")
                            nc.vector.tensor_copy(den, pav[HD:HD + 1, :])
                            nc.vector.reciprocal_approx_fast(rec, den)
                        else:
                            nc.vector.reciprocal(rec, pav[HD:HD + 1, :])
                        recb = recb_pool.tile([HD, 512], F32, tag="recb",
                                              name=f"recb{b}_{h}_{qw}")
                        nc.gpsimd.partition_broadcast(recb, rec)
                        nc.vector.tensor_mul(
                            o_t[r0:r0 + HD, qw * 512:(qw + 1) * 512],
                            pav[0:HD, :], recb)

        def proj_ln2(tt, xt):
            b = tt // (NTT // 2)
            ttl = tt % (NTT // 2)
            yt = ystage_pool.tile([128, D], F32, tag="yst", name=f"yst{tt}")
            for n0, nsz in ((0, 512), (512, 256)):
                ps = psum.tile([128, nsz], F32, tag="mm", name=f"pj{tt}_{n0}")
                for kc in range(KC):
                    nc.tensor.matmul(
                        ps,
                        lhsT=attn_oT[b][kc][:, ttl * 128:(ttl + 1) * 128],
                        rhs=pw_sb[kc][:, n0:n0 + nsz],
                        start=(kc == 0), stop=(kc == KC - 1))
                nc.vector.tensor_add(yt[:, n0:n0 + nsz], ps, xt[:, n0:n0 + nsz])
            eng = nc.sync if tt % 2 == 0 else nc.gpsimd
            eng.dma_start(out=y_scr[tt * 128:(tt + 1) * 128, :], in_=yt)
            layernorm_tile(yt, ln2_tmp, ln2_out, h2T_b[b], ttl, "b")

        # ---------- emission: B0 C0 B1 | proj pools | C1 | proj+LN2 | MLP ---
        phase_b(0)
        attention(0)
        phase_b(1)
        wv_pool.release()
        wqk_pool.release()
        h1T_pool.release()
        vaug0_pool.release()
        qkT0_pool.release()

        # fc1 weights + g early on the left: fc1 of batch 0 backfills PE
        # under batch-1 attention (w1 DMA runs during batch-0 attention)
        w1_pool = tc.alloc_tile_pool(name="wfc1", bufs=1, side="left")
        w1_big = w1_pool.tile([128, KC * HID], BF, name="w1_big")
        nc.sync.dma_start(
            out=w1_big.rearrange("p (c d) -> p c d", d=HID),
            in_=fc1_wT.rearrange("(c p) d -> p c d", p=128))
        w1_sb = [w1_big[:, kc * HID:(kc + 1) * HID] for kc in range(KC)]

        # batch-1 attn output + proj/LN2 pools (left) so proj0 runs under C1
        oT1_pool = tc.alloc_tile_pool(name="oT1", bufs=KC, side="left")
        attn_oT[1] = [oT1_pool.tile([128, N], BF, tag="oT1", name=f"oT1_{i}")
                      for i in range(KC)]
        wp_pool = tc.alloc_tile_pool(name="wproj", bufs=1, side="left")
        pw_big = wp_pool.tile([128, KC * D], BF, name="wp_big")
        nc.sync.dma_start(
            out=pw_big.rearrange("p (c d) -> p c d", d=D),
            in_=proj_wT.rearrange("(c p) d -> p c d", p=128))
        pw_sb = [pw_big[:, kc * D:(kc + 1) * D] for kc in range(KC)]
        ystage_pool = tc.alloc_tile_pool(name="ystage", bufs=3, side="left")
        xr_pool = tc.alloc_tile_pool(name="x_res", bufs=2, side="left")
        ln2_tmp = tc.alloc_tile_pool(name="ln2_tmp", bufs=8, side="left")
        ln2_out = tc.alloc_tile_pool(name="ln2_out", bufs=4, side="left")

        def proj_group(g):
            xg = xr_pool.tile([128, 2, D], F32, tag="xres", name=f"xrg{g}")
            eng = nc.sync if g % 2 == 0 else nc.gpsimd
            eng.dma_start(
                out=xg,
                in_=x_in[g * 256:(g + 1) * 256, :]
                .rearrange("(t p) d -> p t d", p=128))
            # x + proj_b on the idle GpSimd engine, off the critical DVE path
            for t in range(2):
                if F_GP_OPS:
                    nc.gpsimd.tensor_add(xg[:, t, :], xg[:, t, :], projb_sb)
                else:
                    nc.vector.tensor_add(xg[:, t, :], xg[:, t, :], projb_sb)
            for t in range(2):
                proj_ln2(g * 2 + t, xg[:, t, :])

        # batch-0 proj/LN2 interleaved into attention(1) emission: the static
        # PE stream then alternates attention and proj, so the 10-15us exp
        # dependency stalls are filled with proj/transpose work.
        attention(1, filler=lambda p: proj_group(p - 2) if p >= 2 else None)

        vaug1_pool.release()
        qkT1_pool.release()
        recb_pool.release()
        rec_pool.release()
        at_pool.release()

        # batch-1 h2T + fc2 weights (right, in freed attention space)
        h2T1_pool = tc.alloc_tile_pool(name="h2T1", bufs=KC, side="right")
        h2T_b[1] = [h2T1_pool.tile([128, N], BF, tag="h2T1", name=f"h2T1_{i}")
                    for i in range(KC)]
        w2_pool = tc.alloc_tile_pool(name="wfc2", bufs=1, side="right")
        w2_big = w2_pool.tile([128, NOF1 * D], BF, name="w2_big")
        nc.sync.dma_start(
            out=w2_big.rearrange("p (c d) -> p c d", d=D),
            in_=fc2_wT.rearrange("(c p) d -> p c d", p=128))
        w2_sb = [w2_big[:, c * D:(c + 1) * D] for c in range(NOF1)]

        for g in range(4, NTT // 2):
            proj_group(g)

        # Gate the gelus (and nothing else) behind the last LN2 rstd: gelu's
        # bias reads fc1b_g, whose def-chain touches the final rstd tile, so
        # no gelu can be scheduled on ACT before every Ln/Exp is done -> the
        # gelu table set is loaded exactly once.
        junk0 = singles.tile([128, 1], F32)
        nc.vector.tensor_scalar(junk0, last_rstd[0], 0.0, 0.0,
                                ALU.mult, ALU.add)
        fc1b_g = singles.tile([128, NOF1], F32)
        nc.vector.tensor_scalar(fc1b_g, fc1b_sb, junk0, 0.0,
                                ALU.add, ALU.add)

        ln2_out.release()
        ln2_tmp.release()
        xr_pool.release()
        ystage_pool.release()
        wp_pool.release()
        oT1_pool.release()

        g_pool = tc.alloc_tile_pool(name="g", bufs=NOF1 + 4, side="right")
        out_pool = tc.alloc_tile_pool(name="out", bufs=3, side="right")

        # ---------- MLP ----------
        for tw in range(NTW):
            b = tw // 2
            tww = tw % 2  # window within batch
            h2T = h2T_b[b]
            gts = [g_pool.tile([128, 512], BF, tag="g", name=f"g{tw}_{i}")
                   for i in range(NOF1)]
            for of in range(NOF1):
                ps = psum.tile([128, 512], F32, tag="sc", bufs=2,
                               name=f"f1_{tw}_{of}")
                for kc in range(KC):
                    nc.tensor.matmul(
                        ps,
                        lhsT=w1_sb[kc][:, of * 128:(of + 1) * 128],
                        rhs=h2T[kc][:, tww * 512:(tww + 1) * 512],
                        start=(kc == 0), stop=(kc == KC - 1))
                nc.scalar.activation(gts[of], ps, AF.Gelu,
                                     bias=fc1b_g[:, of:of + 1])
            for tl in range(4):
                tt = tw * 4 + tl
                o_sb = out_pool.tile([128, D], F32, tag="o", name=f"o{tt}")
                eng = nc.sync if tt % 2 == 0 else nc.gpsimd
                eng.dma_start(out=o_sb, in_=y_scr[tt * 128:(tt + 1) * 128, :])
                if F_GP_OPS:
                    nc.gpsimd.tensor_add(o_sb, o_sb, fc2b_sb)
                else:
                    nc.vector.tensor_add(o_sb, o_sb, fc2b_sb)
                for n0, nsz in ((0, 512), (512, 256)):
                    ps = psum.tile([128, nsz], F32, tag="mm", name=f"f2_{tt}_{n0}")
                    for c in range(NOF1):
                        nc.tensor.matmul(
                            ps,
                            lhsT=gts[c][:, tl * 128:(tl + 1) * 128],
                            rhs=w2_sb[c][:, n0:n0 + nsz],
                            start=(c == 0), stop=(c == NOF1 - 1))
                    nc.vector.tensor_add(o_sb[:, n0:n0 + nsz], ps,
                                         o_sb[:, n0:n0 + nsz])
                eng2 = nc.gpsimd if tt % 2 == 0 else nc.sync
                eng2.dma_start(out=y_out[tt * 128:(tt + 1) * 128, :],
                               in_=o_sb)

        out_pool.release()
        g_pool.release()
        w2_pool.release()
        h2T1_pool.release()
        w1_pool.release()
        h2T0_pool.release()
        oT0_pool.release()
        psum.release()

    nc.compile()
    return nc


def _get_program(has_qkv_bias):
    key = bool(has_qkv_bias)
    if key not in _PROG_CACHE:
        _PROG_CACHE[key] = _build_program(key)
    return _PROG_CACHE[key]


def kernel(x, qkv_w, proj_w, proj_b, fc1_w, fc1_b, fc2_w, fc2_b,
           norm1_g, norm1_b, norm2_g, norm2_b):
    from concourse.bass_utils import run_bass_kernel_spmd

    x = np.asarray(x, dtype=np.float32)
    qkv_w = np.asarray(qkv_w, dtype=np.float32)
    proj_w = np.asarray(proj_w, dtype=np.float32)
    fc1_w = np.asarray(fc1_w, dtype=np.float32)
    fc2_w = np.asarray(fc2_w, dtype=np.float32)

    bf = ml_dtypes.bfloat16
    scale = HD ** (-0.5)

    # fold LN1 gamma into qkv_w columns; LN1 beta becomes a qkv bias.
    w_eff = qkv_w * np.asarray(norm1_g, np.float32)[None, :]
    b_eff = qkv_w @ np.asarray(norm1_b, np.float32)
    # fold the attention scale into q
    w_eff = w_eff.copy()
    w_eff[:D] *= scale
    b_eff = b_eff.copy()
    b_eff[:D] *= scale
    has_qkv_bias = bool(np.any(b_eff != 0.0))

    qk_wT = np.ascontiguousarray(w_eff[:2 * D].T, dtype=bf)
    v_wT = np.ascontiguousarray(w_eff[2 * D:].T, dtype=bf)
    proj_wT = np.ascontiguousarray(proj_w.T, dtype=bf)
    # fold LN2 gamma into fc1_w columns; LN2 beta into fc1 bias.
    fc1_eff = fc1_w * np.asarray(norm2_g, np.float32)[None, :]
    fc1_b_eff = np.asarray(fc1_b, np.float32) + fc1_w @ np.asarray(norm2_b, np.float32)
    fc1_wT = np.ascontiguousarray(fc1_eff.T, dtype=bf)
    fc2_wT = np.ascontiguousarray(fc2_w.T, dtype=bf)

    shared = {
        "qk_wT": qk_wT, "v_wT": v_wT, "proj_wT": proj_wT,
        "fc1_wT": fc1_wT, "fc2_wT": fc2_wT,
        "proj_b": np.ascontiguousarray(proj_b, np.float32),
        "fc1_b": np.ascontiguousarray(fc1_b_eff, np.float32),
        "fc2_b": np.ascontiguousarray(fc2_b, np.float32),
    }
    if has_qkv_bias:
        shared["qk_b"] = np.ascontiguousarray(b_eff[:2 * D], np.float32)
        shared["v_b"] = np.ascontiguousarray(b_eff[2 * D:], np.float32)

    in_maps = []
    for c in range(NCORES):
        xc = np.ascontiguousarray(
            x[c * BC:(c + 1) * BC].reshape(T, D), dtype=np.float32)
        in_maps.append({"x": xc, **shared})

    nc = _get_program(has_qkv_bias)
    res = run_bass_kernel_spmd(nc, in_maps, core_ids=list(range(NCORES)))

    out = np.empty((B, N, D), dtype=np.float32)
    for c in range(NCORES):
        out[c * BC:(c + 1) * BC] = res.results[c]["y"].reshape(BC, N, D)
    return out

